# revision 61
# baseline (speedup 1.0000x reference)
"""Trainium2 Bass kernel for nn_Attention_56633438765665.

Cross-attention with rotary embeddings:
  B=2, N=2048, D=1024, H=16 heads, head_dim=64, fp32 in/out.

Sharding: DP=2 over batch x TP=4 over heads (Megatron-style).
Core c handles batch c//4 and heads [4*(c%4), 4*(c%4)+4).

Epilogue (vs the ReduceScatter-of-projections baseline): output token
ownership is 64-row slices — core c owns rows [ib*512+64c, +64) of
BOTH batches for every 512-token query block ib.  After each block's
softmax normalization the cores exchange the small normalized head
outputs ohT with ONE 8-core mesh AllToAll (256 KB per core, every
part real payload: part j = my 256 head-dims x core j's 64 tokens),
then every core runs the final projection fully locally with the FULL
Wout.  This replaces a 36 us, 1 MB-input ReduceScatter per block with
a ~7 us AllToAll and shrinks the exposed tail after the last block.

Device dataflow (per core; "T" = transposed layout, tokens on the free
dim, feature dims on partitions):
  qT [256, 2048] = Wq_loc^T @ x^T      (PE, streamed x^T chunks)
  rotary: q'T = qT*cos + (R2 @ qT)*sin (one extra PE matmul + DVE;
      R2 = block-diag rotate_half matrix prepared on the host)
  k'T and v^T share one streamed pass over context^T; v^T is
      PE-transposed into the natural [token, head*65] layout with a
      memset ones column per head (softmax denominators)
  per 512-wide query block, head pair chunk, fused j-loop:
    scoresT[j,i] for both heads side by side in one 2-bank PSUM tile
    (row-packed K=64 matmul pair, executed concurrently in the two
    64-row PE groups) -> ONE exp on ACT [128, 1024]
    out_headT_aug [65, i] += v_aug^T @ expT  (row 64 = softmax sums)
  normalize (no transposes): copy accumulators to SBUF (frees PSUM
    for the next j-loop immediately), DVE reciprocal of the sums row
    -> gpsimd partition_broadcast across the 64 head-dims -> one DVE
    multiply per head writes normalized ohT (cross-partition write
    puts head 1 at partitions 64-127)
  stage ohT slices -> AllToAll -> gather -> local final projection
    [128 owned tokens (64 per batch), 1024] with full Wout (fp16)

Scheduling: the j-loop is ACT(exp)-bound (~1.27 us/iteration vs ~0.9
us of PE work), so the next block's q-projection — split into single-
matmul sub-items with the x chunk DMA prefetched one item ahead — and
its rotary drip one-per-iteration INTO the loop's exp-wait slack.
The dripped final projections lag their AllToAll by >=2 blocks
(block 2 runs proj 0, block 3 runs proj 1+2 overlapping the last
exchange's flight, the tail runs proj 3): inter-core skew of 10-15 us
(cores throttle differently) makes any closer placement stall the
in-order tensor queue on collective completion.  Queue discipline:
x/context streams + gathers + output DMAs on sync (plus block-0 odd
chunks on scalar), weight chunks + cos/sin on gpsimd ordered so the
head-critical x chunks own the early SDMA bandwidth, and gpsimd
otherwise carries only bcast -> stage -> collective per block so a
collective trigger never queues behind a wait on a previous
collective.  A tiny warmup AllToAll right after the framework's
prelude barrier absorbs most of the initial start skew.

Matmul inputs are fp16: the PE streams 1 row/cycle (like bf16) but
keeps 10 mantissa bits, and every value in this problem fits fp16
range.  PSUM accumulation is fp32; softmax normalization is fp32
with fp16 denominator reciprocals (~5e-4 rel error).  No softmax
max-subtraction (shift-invariant, exp() stays in range).

The rotary scratch matmul writes an "acc"-ring PSUM tile (the slot
its own q-raw read just freed), keeping the j-loop's "sc" score-tile
ring a pure double-buffer — without this, each block boundary stalls
~7 us on the PSUM ring chained behind the norm broadcast.  Matmul
outputs must fit one 512-float PSUM bank, so all projection matmuls
stream 512-wide halves.

The kv phase is otherwise DVE-bound (~8.7 us/n-block of copies and
rotary muls vs ~7 us of PE work), so the vT copies ride the Scalar
engine (idle before the first exp) and the rotary q-raw copy heads
the DVE queue.  At each normalization the two accumulator copies run
in parallel on Scalar and Vector.

Measured on 8 axon trn2 NeuronCores: ~293-295 us HW exec on clean
runs (vs ~356 us baseline), relative error ~1.4e-3 vs the fp32 jax
reference.  Runs where one peer core is hard-throttled (instruction
timings show core 0 at full speed but collectives waiting 40-60 us)
reach ~340-365 us; that is environmental and shows up for any kernel
with cross-core dependencies.
"""

import ml_dtypes
import numpy as np

import concourse.bass as bass
import concourse.mybir as mybir
import concourse.tile as tile
from concourse import bacc
from concourse.bass_utils import run_bass_kernel_spmd

F32 = mybir.dt.float32
BF16 = mybir.dt.bfloat16
FP16 = mybir.dt.float16

B, N, D, H, HD = 2, 2048, 1024, 16, 64
NCORES, TP = 8, 4
H_LOC = H // TP          # 4 heads per core
NCH = H_LOC // 2         # 2 chunks of 2 heads (128 partitions)
KCH = D // 128           # 8 contraction chunks for the projections
NQ = N // 512            # 4 token 512-blocks
NJ = N // 128            # 16 key tiles
NSCAT = N // TP          # 512 output rows per core (128/block, 64/batch)

MM_DT = FP16             # matmul input dtype
NP_IN = {BF16: ml_dtypes.bfloat16, FP16: np.float16}.get(MM_DT, np.float32)


def build_nc():
    nc = bacc.Bacc(None, target_bir_lowering=False)

    xT = nc.dram_tensor("xT", [D, N], MM_DT, kind="ExternalInput")
    cT = nc.dram_tensor("cT", [D, N], MM_DT, kind="ExternalInput")
    cos2 = nc.dram_tensor("cos2", [128, N], MM_DT, kind="ExternalInput")
    sin2 = nc.dram_tensor("sin2", [128, N], MM_DT, kind="ExternalInput")
    r2t = nc.dram_tensor("r2t", [128, 128], MM_DT, kind="ExternalInput")
    ident = nc.dram_tensor("ident", [128, 128], F32, kind="ExternalInput")
    wq = nc.dram_tensor("wq", [D, 256], MM_DT, kind="ExternalInput")
    wk = nc.dram_tensor("wk", [D, 256], MM_DT, kind="ExternalInput")
    wv = nc.dram_tensor("wv", [D, 256], MM_DT, kind="ExternalInput")
    wout = nc.dram_tensor("wout", [D, D], MM_DT, kind="ExternalInput")
    out = nc.dram_tensor("out", [NSCAT, D], MM_DT, kind="ExternalOutput")

    Exp = mybir.ActivationFunctionType.Exp
    Copy = mybir.ActivationFunctionType.Copy

    with tile.TileContext(nc) as tc:
        with (
            tc.tile_pool(name="const", bufs=1) as constp,
            tc.tile_pool(name="stream", bufs=1) as strp,
            tc.tile_pool(name="persist", bufs=1) as pp,
            tc.tile_pool(name="exp", bufs=1) as expp,
            tc.tile_pool(name="tmp", bufs=1) as tmpp,
            tc.tile_pool(name="psum", bufs=1, space="PSUM") as psp,
            tc.tile_pool(name="dram", bufs=1, space="DRAM") as dramp,
        ):
            # ---- persistent activation buffers ----
            # q^T is double-buffered by block parity: the dripped rotary
            # writes block ib+1's qp while scores still read block ib's —
            # one shared tile makes the framework's conservative WAR
            # tracking chain the rotary muls behind in-flight score reads
            # (a recurring ~5 us stall at each block start)
            qp2 = [pp.tile([128, NCH, N], MM_DT, tag=f"qT{i}", name=f"qp{i}")
                   for i in range(2)]
            kp = pp.tile([128, NCH, N], MM_DT, tag="kT")    # rotated k^T
            vt = pp.tile([128, NJ, 4 * (HD + 1)], MM_DT, tag="v")
            oh = pp.tile([128, NCH, N], MM_DT, tag="ohT")   # normalized heads^T

            # ---- weights / constants; chunked DMAs spread over queues ----
            wq_s = constp.tile([128, KCH, 256], MM_DT, tag="wq_s")
            wk_s = constp.tile([128, KCH, 256], MM_DT, tag="wk_s")
            wv_s = constp.tile([128, KCH, 256], MM_DT, tag="wv_s")
            wout_s = constp.tile([128, KCH, D], MM_DT, tag="wout_s")
            cos_s = constp.tile([128, N], MM_DT, tag="cos_s")
            sin_s = constp.tile([128, N], MM_DT, tag="sin_s")
            r2t_s = constp.tile([128, 128], MM_DT, tag="r2t_s")
            ident_s = constp.tile([128, 128], F32, tag="ident_s")
            ones_s = constp.tile([2, 128], MM_DT, tag="ones_s")
            nc.vector.memset(ones_s[:], 0.0)
            nc.vector.memset(ones_s[0:1, :], 1.0)

            # the one pair the first matmul needs, on the stream queue
            nc.sync.dma_start(wq_s[:, 0, :], wq[0:128, :])

            def q_proj_items(n, split=False):
                """Work items (closures) computing q'T for token block n;
                emitted piecewise so they interleave with attention.  With
                split=True the matmuls are single-matmul sub-items sized to
                hide inside one j-iteration's ACT(exp) slack, with the x
                chunk DMA prefetched one sub-item ahead."""
                ns = slice(n * 512, (n + 1) * 512)
                pss = []
                sts = {}

                def fetch(d):
                    st = strp.tile([128, 512], MM_DT, tag="xs", bufs=24,
                                   name=f"xs_{n}_{d}")
                    # split dispatches across two queues: the 8-deep
                    # dispatch train on one queue costs a constant ~5.2 us
                    # (650 ns each) that surfaces as a tensor stall.  Block
                    # 0 pairs sync+scalar (scalar idle pre-exp); dripped
                    # blocks pair sync+gpsimd (gpsimd idle between norm
                    # broadcasts, and fetches wait nothing so they cannot
                    # chain a collective)
                    if n == 0:
                        eng = nc.scalar if d % 2 == 1 else nc.sync
                    else:
                        eng = nc.gpsimd if d % 2 == 1 else nc.sync
                    eng.dma_start(st[:], xT[d * 128:(d + 1) * 128, ns])
                    sts[d] = st

                def mm(d, m):
                    if d == 0 and m == 0:
                        for mm_ in range(NCH):
                            pss.append(psp.tile(
                                [128, 512], F32, tag="acc", bufs=4,
                                name=f"pj_{n}_{mm_}"))
                    nc.tensor.matmul(
                        pss[m][:],
                        lhsT=wq_s[:, d, m * 128:(m + 1) * 128],
                        rhs=sts[d][:],
                        start=(d == 0),
                        stop=(d == KCH - 1),
                    )

                def chunk(d):
                    if d not in sts:
                        fetch(d)
                    for m in range(NCH):
                        if d == 0 and m == 0:
                            mm(0, 0)
                        else:
                            nc.tensor.matmul(
                                pss[m][:],
                                lhsT=wq_s[:, d, m * 128:(m + 1) * 128],
                                rhs=sts[d][:],
                                start=(d == 0),
                                stop=(d == KCH - 1),
                            )

                def prefetch_all():
                    for d in range(KCH):
                        fetch(d)

                qraws, ps_rots = {}, {}

                def rot_mm(m):
                    qraw = tmpp.tile([128, 512], MM_DT, tag="qraw", bufs=3)
                    nc.vector.tensor_copy(qraw[:], pss[m][:])
                    # "acc" ring (not "sc"): the slot freed by this rot's
                    # own pj read; keeps the j-loop's score-tile ring free
                    ps_rot = psp.tile([128, 512], F32, tag="acc", bufs=4,
                                      name="ps_rot")
                    nc.tensor.matmul(
                        ps_rot[:], lhsT=r2t_s[:], rhs=qraw[:],
                        start=True, stop=True,
                    )
                    qraws[m], ps_rots[m] = qraw, ps_rot

                def rot_fin(m):
                    qp = qp2[n % 2]
                    nc.vector.tensor_mul(qp[:, m, ns], qraws[m][:],
                                         cos_s[:, ns])
                    tsin = tmpp.tile([128, 512], F32, tag="tsin", bufs=3)
                    nc.vector.tensor_mul(tsin[:], ps_rots[m][:], sin_s[:, ns])
                    nc.vector.tensor_add(qp[:, m, ns], qp[:, m, ns], tsin[:])

                def rot(m):
                    rot_mm(m)
                    rot_fin(m)

                rots = [lambda m=m: rot(m) for m in range(NCH)]
                if split:
                    # rots ride the drip too (slots 17-20 of the j-loop),
                    # split so BOTH qraw copies head the DVE queue before
                    # the rotary muls: emitting rot(m0) whole makes rot
                    # (m1)'s matmul wait its qraw copy stuck behind m0's
                    # muls (which wait m0's matmul) — a ~5 us cross-engine
                    # ping-pong stalling the in-order tensor queue
                    items = [prefetch_all]
                    items += [lambda d=d, m=m: mm(d, m)
                              for d in range(KCH) for m in range(NCH)]
                    items += [lambda m=m: rot_mm(m) for m in range(NCH)]
                    items += [lambda m=m: rot_fin(m) for m in range(NCH)]
                    rots = []
                else:
                    items = [prefetch_all]
                    items += [lambda d=d: chunk(d) for d in range(KCH)]
                return items, rots

            # q' block 0: first x chunk next on the stream queue, then the
            # remaining weight chunks flow on the idle engine queues
            _items0, _rots0 = q_proj_items(0)
            _items0[0]()
            for d in range(1, KCH):
                nc.gpsimd.dma_start(wq_s[:, d, :], wq[d * 128:(d + 1) * 128, :])
            for d in range(KCH):
                nc.gpsimd.dma_start(wk_s[:, d, :], wk[d * 128:(d + 1) * 128, :])
            # cos/sin then wv ride the sync queue behind the x0 chunks,
            # keeping early SDMA bandwidth for the head-critical x stream;
            # cos is needed by q0's rotary at ~20 us, wv only at ~25 us
            nc.sync.dma_start(cos_s[:], cos2[:, :])
            nc.sync.dma_start(sin_s[:], sin2[:, :])
            for d in range(KCH):
                nc.sync.dma_start(wv_s[:, d, :], wv[d * 128:(d + 1) * 128, :])
            nc.scalar.dma_start(ident_s[:], ident[:, :])
            nc.scalar.dma_start(r2t_s[:], r2t[:, :])
            vt_ones = vt.rearrange("p j (h c) -> p j h c", c=HD + 1)[:, :, :, 64]
            nc.vector.memset(vt_ones, 1.0)
            # warmup AllToAll: re-syncs the cores right after the framework's
            # prelude barrier so block 0's real exchange isn't hit with the
            # accumulated start skew (~10 us on the first collective)
            wu_in = dramp.tile([8, 64], MM_DT, name="wu_in")
            wu_out = dramp.tile([8, 64], MM_DT, name="wu_out")
            nc.gpsimd.dma_start(wu_in[:], xT[0:8, 0:64])
            nc.gpsimd.collective_compute(
                "AllToAll",
                mybir.AluOpType.bypass,
                replica_groups=[list(range(NCORES))],
                ins=[wu_in[:].opt()],
                outs=[wu_out[:].opt()],
            )
            for it in _items0[1:]:
                it()
            for r in _rots0:
                r()

            # ---- k' and v share one streamed pass over context^T ----
            for n in range(NQ):
                ns = slice(n * 512, (n + 1) * 512)
                pss = [
                    psp.tile([128, 512], F32, tag="acc", bufs=4,
                             name=f"pk_{n}_{m}")
                    for m in range(NCH)
                ]
                ps_vT = [
                    psp.tile([128, 512], F32, tag="acc", bufs=4,
                             name=f"pv_{n}_{m}")
                    for m in range(NCH)
                ]
                for d in range(KCH):
                    st = strp.tile([128, 512], MM_DT, tag="xs", bufs=24,
                                   name=f"cs_{n}_{d}")
                    nc.sync.dma_start(st[:], cT[d * 128:(d + 1) * 128, ns])
                    for m in range(NCH):
                        nc.tensor.matmul(
                            pss[m][:],
                            lhsT=wk_s[:, d, m * 128:(m + 1) * 128],
                            rhs=st[:],
                            start=(d == 0),
                            stop=(d == KCH - 1),
                        )
                    for m in range(NCH):
                        nc.tensor.matmul(
                            ps_vT[m][:],
                            lhsT=wv_s[:, d, m * 128:(m + 1) * 128],
                            rhs=st[:],
                            start=(d == 0),
                            stop=(d == KCH - 1),
                        )
                # rot chain first (qraw at the head of the DVE queue) and
                # vT copies on the Scalar engine (idle before the first
                # exp): the kv phase is otherwise DVE-bound at ~8.7 us per
                # n-block vs ~7 us of PE work
                for m in range(NCH):
                    qraw = tmpp.tile([128, 512], MM_DT, tag="qraw", bufs=3)
                    nc.vector.tensor_copy(qraw[:], pss[m][:])
                    # "acc" ring (not "sc"): the slot freed by this rot's
                    # own pj read; keeps the j-loop's score-tile ring free
                    ps_rot = psp.tile([128, 512], F32, tag="acc", bufs=4,
                                      name="ps_rot")
                    nc.tensor.matmul(
                        ps_rot[:], lhsT=r2t_s[:], rhs=qraw[:],
                        start=True, stop=True,
                    )
                    nc.vector.tensor_mul(kp[:, m, ns], qraw[:], cos_s[:, ns])
                    tsin = tmpp.tile([128, 512], F32, tag="tsin", bufs=3)
                    nc.vector.tensor_mul(tsin[:], ps_rot[:], sin_s[:, ns])
                    nc.vector.tensor_add(kp[:, m, ns], kp[:, m, ns], tsin[:])
                for m in range(NCH):
                    vT_sb = tmpp.tile([128, 512], F32, tag="vT_sb", bufs=2)
                    nc.scalar.activation(vT_sb[:], ps_vT[m][:], Copy)
                    ps_vt = psp.tile([128, 1024], F32, tag="sc", bufs=2,
                                     name="ps_vt")
                    for jj in range(4):
                        nc.tensor.transpose(
                            ps_vt[:, jj * 128:(jj + 1) * 128],
                            vT_sb[:, jj * 128:(jj + 1) * 128],
                            ident_s[:, :],
                        )
                    for jj in range(4):
                        j = n * 4 + jj
                        dstv = vt[:, j, :].rearrange(
                            "p (h c) -> p h c", c=HD + 1)
                        srcv = ps_vt[:, jj * 128:(jj + 1) * 128].rearrange(
                            "p (h c) -> p h c", c=HD)
                        nc.scalar.activation(
                            dstv[:, 2 * m:2 * m + 2, 0:HD], srcv[:], Copy)

            # wout lands on the stream queue after the k/v context chunks:
            # first needed by block 0's projection (~130 us), far behind
            for d in range(KCH):
                nc.sync.dma_start(
                    wout_s[:, d, :], wout[d * 128:(d + 1) * 128, :])

            # ---- AllToAll exchange + local final projection helpers ----
            a2a_out = {}

            def exchange(ib):
                """Stage ohT slices for block ib and fire one AllToAll.
                a2a_in part j = [(t p), 64] = my 256 head-dims x core j's
                64 owned tokens of block ib (batch-symmetric: every part
                is real payload for its destination).  One op per block:
                more frequent, smaller collectives measured SLOWER here —
                they chain through the in-order gpsimd queue and pay the
                cross-core sync cost once per op."""
                isl = slice(ib * 512, (ib + 1) * 512)
                a_in = dramp.tile([2048, 64], MM_DT, name=f"a2ain_{ib}")
                a_out = dramp.tile([2048, 64], MM_DT, name=f"a2aout_{ib}")
                a_in_v = a_in.rearrange("(j t p) f -> p t j f", t=2, p=128)
                for t in range(2):
                    nc.gpsimd.dma_start(
                        a_in_v[:, t],
                        oh[:, t, isl].rearrange("p (j f) -> p j f", j=8),
                    )
                nc.gpsimd.collective_compute(
                    "AllToAll",
                    mybir.AluOpType.bypass,
                    replica_groups=[list(range(NCORES))],
                    ins=[a_in[:].opt()],
                    outs=[a_out[:].opt()],
                )
                a2a_out[ib] = a_out

            def proj_items(ib):
                """Work items: gather block ib's AllToAll result and run
                the local final projection for the 128 owned token rows
                (64 of each batch), then DMA them out."""
                # ohf[p, t, m, h, f]: global contraction chunk kc=2m+t,
                # token column = h*64+f (h = batch half)
                ohf = tmpp.tile([128, 2, 4, 2, 64], MM_DT, tag="ohf", bufs=2)
                src = a2a_out[ib].rearrange(
                    "(h m t p) f -> p t m h f", h=2, m=4, t=2, p=128)

                def gather(t):
                    # sync queue: keeps a2a-completion waits OFF gpsimd so
                    # the norm broadcasts / next collective never chain
                    # behind a previous exchange
                    for h in range(2):
                        nc.sync.dma_start(ohf[:, t, :, h], src[:, t, :, h])

                pf = []

                def proj(half):
                    ps = psp.tile([128, 512], F32, tag="acc", bufs=4,
                                  name=f"pf_{ib}_{half}")
                    pf.append(ps)
                    for kc in range(KCH):
                        m, t = kc // 2, kc % 2
                        nc.tensor.matmul(
                            ps[:],
                            lhsT=ohf[:, t, m].rearrange("p h f -> p (h f)"),
                            rhs=wout_s[:, kc, half * 512:(half + 1) * 512],
                            start=(kc == 0),
                            stop=(kc == KCH - 1),
                        )

                def emit(half):
                    fo = tmpp.tile([128, 512], MM_DT, tag="fo", bufs=3)
                    nc.vector.tensor_copy(fo[:], pf[half][:])
                    nc.sync.dma_start(
                        out[ib * 128:(ib + 1) * 128,
                            half * 512:(half + 1) * 512], fo[:])

                # proj(h) must be followed by emit(h) before the next
                # j-loop allocates its PSUM accumulators, else the 4-buf
                # "acc" pool deadlocks against the in-order engine queues
                return [lambda t=t: gather(t) for t in range(2)] + \
                       [lambda: proj(0), lambda: emit(0),
                        lambda: proj(1), lambda: emit(1)]

            # ---- attention per query block; block ib+1's q-projection
            # ---- matmuls drip one-per-j-iteration INTO the ACT-bound
            # ---- j-loop (the PE has ~400 ns of exp-wait slack per iter);
            # ---- block ib-1's projection fills the post-norm windows
            for ib in range(NQ):
                isl = slice(ib * 512, (ib + 1) * 512)
                if ib + 1 < NQ:
                    jdrip, qrots = q_proj_items(ib + 1, split=True)
                    # fire the x prefetch NOW (block start): dispatched at a
                    # drip slot it lands mid-block, colliding with the
                    # previous exchange's SDMA traffic (~16 us late x
                    # chunks observed); here the transfers clear before the
                    # collective fires
                    jdrip.pop(0)()
                else:
                    jdrip, qrots = [], []
                # lag the dripped projections >=2 blocks behind their
                # AllToAll (block 2 runs proj 0; block 3 runs proj 1+2,
                # overlapping the final exchange's flight): with ~10-15 us
                # of inter-core skew a lag-1 window still stalls the
                # in-order tensor queue on collective completion
                window = []
                if ib == 2:
                    window = proj_items(0)
                elif ib == 3:
                    window = proj_items(1) + proj_items(2)
                deferred_muls = []
                for t in range(NCH):
                    ps_oh = [
                        psp.tile([HD + 1, 512], F32, tag="acc", bufs=4,
                                 name=f"oh_{t}_{ib}_{hh}")
                        for hh in range(2)
                    ]
                    # scores are software-pipelined TWO iterations ahead:
                    # the critical cycle exp(j) -> attnv(j) -> scores(j+2)
                    # -> exp(j+2) then spans two exp slots, so the ACT
                    # engine runs exps back-to-back (the in-order tensor
                    # queue otherwise serializes scores(j+1) behind
                    # attnv(j), costing ~150 ns per iteration).  scores
                    # (j+2) reuses the ps_s buffer exp(j) just released,
                    # so the 2-deep "sc" ring still suffices.
                    ps_sd = {}

                    def scores(j):
                        ps_s = psp.tile([128, 1024], F32, tag="sc", bufs=2,
                                        name="ps_s")
                        for hh in range(2):
                            rows = slice(hh * 64, (hh + 1) * 64)
                            nc.tensor.matmul(
                                ps_s[:, hh * 512:(hh + 1) * 512],
                                lhsT=kp[rows, t, j * 128:(j + 1) * 128],
                                rhs=qp2[ib % 2][rows, t, isl],
                                start=True, stop=True,
                            )
                        ps_sd[j] = ps_s

                    scores(0)
                    scores(1)
                    for j in range(NJ):
                        et = expp.tile([128, 1024], MM_DT, tag="expT", bufs=12)
                        nc.scalar.activation(et[:], ps_sd.pop(j)[:], Exp)
                        for hh in range(2):
                            h_loc = t * 2 + hh
                            vcols = slice(h_loc * (HD + 1), (h_loc + 1) * (HD + 1))
                            nc.tensor.matmul(
                                ps_oh[hh][:],
                                lhsT=vt[:, j, vcols],
                                rhs=et[:, hh * 512:(hh + 1) * 512],
                                start=(j == 0),
                                stop=(j == NJ - 1),
                            )
                        if j + 2 < NJ:
                            scores(j + 2)
                        if j == 6 and deferred_muls:
                            deferred_muls.pop(0)()
                        if jdrip:
                            jdrip.pop(0)()
                    # normalize in the transposed layout: copy the two
                    # accumulators to SBUF right away (frees their PSUM
                    # banks so the next j-loop starts ~4 us sooner), take
                    # per-head row-sum reciprocals to a [1, 1024] fp16 row,
                    # broadcast across the 64 head-dims with a K=2 matmul
                    # (row 1 of rec_sb is zeroed), and one DVE multiply per
                    # head writes normalized oh directly (no transposes)
                    # the two accumulator copies run in parallel on ACT
                    # (idle right after its last exp) and DVE, halving the
                    # norm chain's first stage
                    aug = tmpp.tile([65, 1024], F32, tag="aug", bufs=2)
                    nc.scalar.activation(aug[:, 0:512], ps_oh[0][:], Copy)
                    nc.vector.tensor_copy(aug[:, 512:1024], ps_oh[1][:])
                    rec_sb = tmpp.tile([1, 1024], MM_DT, tag="recb", bufs=2)
                    with nc.allow_low_precision(
                            reason="fp16 softmax denominators: ~5e-4 rel "
                                   "error, well inside the 2e-2 budget"):
                        nc.vector.reciprocal(
                            rec_sb[0:1, :], aug[64:65, :])
                    bc_sb = tmpp.tile([64, 1024], MM_DT, tag="bcsb", bufs=2)
                    nc.gpsimd.partition_broadcast(bc_sb[:], rec_sb[0:1, :])

                    def norm_muls(t=t, aug=aug, bc_sb=bc_sb):
                        for hh in range(2):
                            nc.vector.tensor_mul(
                                oh[hh * 64:(hh + 1) * 64, t, isl],
                                aug[0:64, hh * 512:(hh + 1) * 512],
                                bc_sb[:, hh * 512:(hh + 1) * 512])

                    if t == 0:
                        # defer t=0's oh writes into the t=1 j-loop: emitted
                        # here they block the in-order DVE queue waiting on
                        # the gpsimd broadcast, which delays the dripped
                        # rot_fin muls and thus the "acc" ring release the
                        # NEXT block's dripped q-projection needs (~5 us
                        # stall); oh isn't read until the stage DMA at the
                        # end of t=1
                        deferred_muls.append(norm_muls)
                    else:
                        norm_muls()
                    # the block's exchange fires right after the t=1
                    # normalization (gpsimd order: bcast -> stage -> cc,
                    # independent of the window drip below); the previous
                    # block's projection runs entirely in the t=1 window,
                    # ~25 us after its AllToAll fired, so nothing stalls
                    # on collective completion
                    if t == 1:
                        exchange(ib)
                        for it in window:
                            it()
                        while jdrip:
                            jdrip.pop(0)()
                        for r in qrots:
                            r()

            # ---- tail: last block's projection (matmul output must fit
            # one PSUM bank, so the 512-wide halves stay) ----
            for it in proj_items(NQ - 1):
                it()

    nc.finalize()  # bacc register allocation; the pjrt path doesn't do it
    return nc


def make_in_maps(x, context, pos_emb, Wq, Wkv, Wout):
    """Host-side sharding: slice weights per core, transpose activations."""
    scale = HD ** -0.5
    cos = np.ascontiguousarray(np.cos(pos_emb).T).astype(np.float32)
    sin = np.ascontiguousarray(np.sin(pos_emb).T).astype(np.float32)
    cos2 = np.concatenate([cos, cos], axis=0)
    sin2 = np.concatenate([sin, sin], axis=0)
    # rotate_half as a matrix: rot = R @ q (per head), block-diag for 2 heads
    R = np.zeros((HD, HD), np.float32)
    R[np.arange(32), np.arange(32) + 32] = -1.0
    R[np.arange(32) + 32, np.arange(32)] = 1.0
    r2t = np.zeros((128, 128), np.float32)
    r2t[:64, :64] = R.T
    r2t[64:, 64:] = R.T
    ident = np.eye(128, dtype=np.float32)

    xTb = [np.ascontiguousarray(x[b].T).astype(NP_IN) for b in range(B)]
    cTb = [np.ascontiguousarray(context[b].T).astype(NP_IN) for b in range(B)]
    wout_full = np.ascontiguousarray(Wout).astype(NP_IN)

    in_maps = []
    for c in range(NCORES):
        b, g = c // TP, c % TP
        cols = slice(256 * g, 256 * (g + 1))
        in_maps.append({
            "xT": xTb[b],
            "cT": cTb[b],
            "cos2": cos2.astype(NP_IN),
            "sin2": sin2.astype(NP_IN),
            "r2t": r2t.astype(NP_IN),
            "ident": ident,
            "wq": (np.ascontiguousarray(Wq[:, cols]) * scale).astype(NP_IN),
            "wk": np.ascontiguousarray(Wkv[:, :D][:, cols]).astype(NP_IN),
            "wv": np.ascontiguousarray(Wkv[:, D:][:, cols]).astype(NP_IN),
            "wout": wout_full,
        })
    return in_maps


def assemble(results, b_out):
    """Core c's out rows for block ib are [ib*128, ib*128+128): the first
    64 are batch-0 rows [ib*512+64c, +64), the next 64 the same rows of
    batch 1."""
    full = np.empty((B, N, D), np.float32)
    for c in range(NCORES):
        o = results[c]["out"]
        for ib in range(NQ):
            rows = slice(ib * 512 + c * 64, ib * 512 + (c + 1) * 64)
            full[0, rows, :] = o[ib * 128:ib * 128 + 64, :].astype(np.float32)
            full[1, rows, :] = o[ib * 128 + 64:ib * 128 + 128, :].astype(
                np.float32)
    return full + b_out.astype(np.float32)


_NC_CACHE = {}


def kernel(x, context, pos_emb, Wq, Wkv, Wout, b_out):
    x = np.asarray(x, np.float32)
    context = np.asarray(context, np.float32)
    pos_emb = np.asarray(pos_emb, np.float32)
    Wq = np.asarray(Wq, np.float32)
    Wkv = np.asarray(Wkv, np.float32)
    Wout = np.asarray(Wout, np.float32)
    b_out = np.asarray(b_out, np.float32)

    if "nc" not in _NC_CACHE:
        _NC_CACHE["nc"] = build_nc()
    nc = _NC_CACHE["nc"]
    in_maps = make_in_maps(x, context, pos_emb, Wq, Wkv, Wout)
    res = run_bass_kernel_spmd(nc, in_maps, core_ids=list(range(NCORES)))
    return assemble(res.results, b_out)


if __name__ == "__main__":
    rng = np.random.default_rng(0)
    inputs = {
        "x": rng.standard_normal((B, N, D)).astype(np.float32),
        "context": rng.standard_normal((B, N, D)).astype(np.float32),
        "pos_emb": rng.standard_normal((N, HD)).astype(np.float32),
        "Wq": (rng.standard_normal((D, D)) * D ** -0.5).astype(np.float32),
        "Wkv": (rng.standard_normal((D, 2 * D)) * D ** -0.5).astype(np.float32),
        "Wout": (rng.standard_normal((D, D)) * D ** -0.5).astype(np.float32),
        "b_out": np.zeros((D,), np.float32),
    }
    out = kernel(**inputs)
    print("kernel output", out.shape, out.dtype, float(np.abs(out).max()))
